# revision 8
# baseline (speedup 1.0000x reference)
"""Trainium2 Bass kernel for nn_CustomLayer_35682588295215.

Math (from the reference):
    W = scatter_add(zeros(4096, 4096), (row_ids, col_idx), values)
    out[b, s, o] = sum_h x[b, s, h] * W[o, h]          # [4, 2048, 4096]

i.e. a dense [8192, 4096] x [4096, 4096]^T GEMM after densifying the
compressed sparse weight.  The scatter is cheap O(nnz) host-side
preprocessing (np.bincount); the 275-GFLOP GEMM runs on 8 NeuronCores.

Sharding: data-parallel over batch*seq (8192 -> 1024 rows per core), the
densified weight replicated.  Per core:
    out_shard[m, n] = sum_k xT[k, m] * Wt[k, n]
with xT = x_shard^T ([4096, 1024]) and Wt = W^T ([4096, 4096]), both laid
out host-side so every DMA is contiguous per partition.

Precision: split-K mixed precision.  The first F8=8 of 32 k-tiles run as
fp8 e4m3 DoubleRow matmuls (2 k-tiles per instruction, 2x PE rate); the
remaining 24 run in bf16 (1 row/cycle, same as fp32r but half the DMA).
Measured rel err 1.89e-2 vs the fp32 reference (gate 2e-2); all psum
accumulation is exact fp32, quantization is host-side RNE, so the value
is deterministic.

Kernel loop (per core): k-outer / m-inner with all 8 PSUM banks holding
the 8 M-tiles of one 512-wide N-block, so each weight element is read
from HBM exactly once.
"""

import sys

for _p in ("/opt/trn_rl_repo",):
    if _p not in sys.path:
        sys.path.insert(0, _p)

import ml_dtypes
import numpy as np

import concourse.bass as bass
import concourse.mybir as mybir
from concourse import bacc, tile
from concourse.bass import ts
from concourse.bass_utils import run_bass_kernel_spmd

N_ROWS = 4096  # output dim (o)
N_COLS = 4096  # input dim (h) = contraction K
B, S = 4, 2048
M_TOT = B * S  # 8192
N_CORES = 8
M = M_TOT // N_CORES  # 1024 rows of x per core

P = 128  # partitions
NB = 512  # N free-dim per PSUM bank
K_TILES = N_COLS // P  # 32
M_TILES = M // P  # 8
N_BLOCKS = N_ROWS // NB  # 8

F8 = 8  # k-tiles computed in fp8 e4m3 DoubleRow (multiple of 2)
F8_GRPS = F8 // 2  # 4 DoubleRow groups (one [P, 2, *] tile pair each)
BF = K_TILES - F8  # 24 k-tiles in bf16
BQ = 2  # bf16 k-tiles per weight DMA
BF_SLOTS = BF // BQ  # 12
WARMUP_MM = 10  # dummy matmuls at t~6.5us so the HAM clock gate is warm
                # (2.4 GHz) before the first real matmul's data lands

FP8_DT = mybir.dt.float8e4
BF16_DT = mybir.dt.bfloat16
NP_FP8 = ml_dtypes.float8_e4m3
NP_BF16 = ml_dtypes.bfloat16

# Filled by run(): max-across-traced-cores HW exec time in ns (None if no trace).
LAST_EXEC_NS = None

_CACHED_NC = None


def _build():
    nc = bacc.Bacc(None, target_bir_lowering=False, debug=False, num_swdge_queues=3)
    # xs8: x_shard^T k-tiles 0..F8 as fp8:   xs8[p, t, m] = x_shard[m, t*128+p]
    # xsb: x_shard^T k-tiles F8..32 as bf16: xsb[p, t, m] = x_shard[m, (F8+t)*128+p]
    xs8_d = nc.dram_tensor("xs8", [P, F8, M], FP8_DT, kind="ExternalInput")
    xsb_d = nc.dram_tensor("xsb", [P, BF, M], BF16_DT, kind="ExternalInput")
    # wt8/wtb: W^T in [p, n, kt, j]: wt[p, n, kt, j] = W[n*512+j, (kt0+kt)*128+p]
    wt8_d = nc.dram_tensor("wt8", [P, N_BLOCKS, F8, NB], FP8_DT, kind="ExternalInput")
    wtb_d = nc.dram_tensor("wtb", [P, N_BLOCKS, BF, NB], BF16_DT, kind="ExternalInput")
    out_d = nc.dram_tensor("out", [M, N_ROWS], mybir.dt.float32, kind="ExternalOutput")

    DR = mybir.MatmulPerfMode.DoubleRow

    # Queue plan: the weight stream (3.5 MiB/block) rides the sync HWDGE
    # path; x-cache loads (7 MiB, block 0 only) and half the output stores
    # ride the gpsimd SWDGE path.  Block 0 moves ~10.5 MiB against two
    # ~184 GB/s queue paths, well inside its ~48 us of compute.
    #
    # wtb bufs=16 lets the sync queue run a full block of weight DMAs ahead
    # of compute; the last block is emitted m-outer/k-inner so its 8 PSUM
    # evictions + 2 MiB of output stores overlap its own matmul stream
    # instead of draining after the final matmul.
    with tile.TileContext(nc) as tc:
        with (
            tc.tile_pool(name="xs8_pool", bufs=F8_GRPS) as xs8_pool,
            tc.tile_pool(name="xsb_pool", bufs=BF_SLOTS) as xsb_pool,
            tc.tile_pool(name="wt8_pool", bufs=2 * F8_GRPS) as wt8_pool,
            tc.tile_pool(name="wtb_pool", bufs=16) as wtb_pool,
            tc.tile_pool(name="out_pool", bufs=10) as out_pool,
            tc.tile_pool(name="psum", bufs=8, space="PSUM") as psum_pool,
        ):
            xs8_t = [None] * F8_GRPS
            xsb_t = [None] * BF_SLOTS

            # PE warmup: zero-matmuls on a memset tile keep the PE busy from
            # ~6.5us (end of NEFF preamble) so the HAM un-throttles to
            # 2.4 GHz while the first real DMAs are still in flight.
            wdum = out_pool.tile([P, NB], BF16_DT, name="wdum", tag="wdum")
            nc.vector.memset(wdum[:], 0.0)
            wps = psum_pool.tile([P, NB], mybir.dt.float32, name="wps", tag="ps")
            for i in range(WARMUP_MM):
                nc.tensor.matmul(
                    wps[:],
                    wdum[:, :P],
                    wdum[:, :],
                    start=(i == 0),
                    stop=(i == WARMUP_MM - 1),
                )

            for n in range(N_BLOCKS):
                last_block = n == N_BLOCKS - 1
                if n == 0:
                    # fp8 x cache: four [P, 2, M] quarters spread across the
                    # gpsimd SWDGE ring and the scalar (Activation) HWDGE
                    # queue so the whole first wave lands in parallel with
                    # the weight stream on sync, and the first DoubleRow
                    # matmul starts ASAP.
                    x_engs = (nc.gpsimd, nc.scalar, nc.gpsimd, nc.gpsimd)
                    for j in range(F8_GRPS):
                        xs8_t[j] = xs8_pool.tile(
                            [P, 2, M], FP8_DT, name="xs8", tag="xs8"
                        )
                        x_engs[j].dma_start(
                            xs8_t[j][:], xs8_d[:, 2 * j : 2 * j + 2, :]
                        )
                w8 = []
                for j in range(F8_GRPS):
                    w8t = wt8_pool.tile([P, 2, NB], FP8_DT)
                    nc.sync.dma_start(w8t[:], wt8_d[:, n, 2 * j : 2 * j + 2, :])
                    w8.append(w8t)
                if n == 0:
                    for t in (0, 1):
                        xsb_t[t] = xsb_pool.tile(
                            [P, BQ, M], BF16_DT, name="xsb", tag="xsb"
                        )
                        nc.gpsimd.dma_start(
                            xsb_t[t][:], xsb_d[:, BQ * t : BQ * t + BQ, :]
                        )
                wb = []
                if last_block:
                    # m-outer block: all 12 bf16 weight tiles up front (the
                    # deep wtb pool prefetched them during block 6).
                    for s in range(BF_SLOTS):
                        wbt = wtb_pool.tile([P, BQ, NB], BF16_DT)
                        nc.sync.dma_start(
                            wbt[:], wtb_d[:, n, BQ * s : BQ * s + BQ, :]
                        )
                        wb.append(wbt)
                psums = [
                    psum_pool.tile([P, NB], mybir.dt.float32, name="ps", tag="ps")
                    for _ in range(M_TILES)
                ]

                def dr_mm(m, j, start):
                    nc.tensor.matmul(
                        psums[m][:],
                        xs8_t[j][:, :, ts(m, P)],
                        w8[j][:, :, :],
                        start=start,
                        stop=False,
                        perf_mode=DR,
                    )

                def bf_mm(m, s, ks, wbt):
                    nc.tensor.matmul(
                        psums[m][:],
                        xsb_t[s][:, ks, ts(m, P)],
                        wbt[:, ks, :],
                        start=False,
                        stop=(s == BF_SLOTS - 1 and ks == BQ - 1),
                    )

                def evict(m):
                    ot = out_pool.tile([P, NB], mybir.dt.float32, name="ot", tag="ot")
                    if m % 2 == 0:
                        nc.vector.tensor_copy(ot[:], psums[m][:])
                    else:
                        nc.scalar.copy(ot[:], psums[m][:])
                    out_eng = nc.gpsimd if m % 2 == 0 else nc.sync
                    out_eng.dma_start(out_d[ts(m, P), ts(n, NB)], ot[:])

                if last_block:
                    for m in range(M_TILES):
                        for j in range(F8_GRPS):
                            dr_mm(m, j, start=(j == 0))
                        for s in range(BF_SLOTS):
                            for ks in range(BQ):
                                bf_mm(m, s, ks, wb[s])
                        evict(m)
                else:
                    # k-outer / m-inner: each weight tile streams from HBM
                    # once and feeds all 8 M-tiles.
                    for j in range(F8_GRPS):
                        for m in range(M_TILES):
                            dr_mm(m, j, start=(j == 0))
                    for s in range(BF_SLOTS):
                        if n == 0 and s + 2 < BF_SLOTS:
                            t = s + 2
                            xsb_t[t] = xsb_pool.tile(
                                [P, BQ, M], BF16_DT, name="xsb", tag="xsb"
                            )
                            nc.gpsimd.dma_start(
                                xsb_t[t][:], xsb_d[:, BQ * t : BQ * t + BQ, :]
                            )
                        wbt = wtb_pool.tile([P, BQ, NB], BF16_DT)
                        nc.sync.dma_start(
                            wbt[:], wtb_d[:, n, BQ * s : BQ * s + BQ, :]
                        )
                        for ks in range(BQ):
                            for m in range(M_TILES):
                                bf_mm(m, s, ks, wbt)
                    # Evictions split across vector+scalar so the 8 PSUM
                    # banks free ~2x sooner at block boundaries.
                    for m in range(M_TILES):
                        evict(m)
    nc.compile()
    return nc


def _get_nc():
    global _CACHED_NC
    if _CACHED_NC is None:
        _CACHED_NC = _build()
    return _CACHED_NC


def _densify_wt(values, col_idx, row_ids):
    # Wt[h, o] = sum of values[i] with col_idx[i] == h, row_ids[i] == o
    idx = col_idx.astype(np.int64) * N_ROWS + row_ids.astype(np.int64)
    wt = np.bincount(idx, weights=values.astype(np.float64), minlength=N_COLS * N_ROWS)
    return wt.astype(np.float32).reshape(N_COLS, N_ROWS)


def _install_ntff_hook():
    """The agent image's antenv package lacks axon_hooks; recreate the tiny
    get/set registry and register the ctypes NTFF hook from trn_agent_boot
    so run_bass_kernel_spmd(trace=True) can capture profiles under axon."""
    import types

    if "antenv.axon_hooks" in sys.modules:
        return
    import antenv
    from trn_agent_boot.trn_boot import _ntff_profile_via_ctypes

    mod = types.ModuleType("antenv.axon_hooks")
    mod._hook = _ntff_profile_via_ctypes("/opt/axon/libaxon_pjrt.so")

    def get_axon_ntff_profile_hook():
        return mod._hook

    def set_axon_ntff_profile_hook(h):
        mod._hook = h

    mod.get_axon_ntff_profile_hook = get_axon_ntff_profile_hook
    mod.set_axon_ntff_profile_hook = set_axon_ntff_profile_hook
    sys.modules["antenv.axon_hooks"] = mod
    antenv.axon_hooks = mod


def kernel(x, values, col_idx, row_ids, trace=False):
    global LAST_EXEC_NS
    if trace:
        _install_ntff_hook()
    x = np.ascontiguousarray(np.asarray(x, dtype=np.float32))
    wt = _densify_wt(np.asarray(values), np.asarray(col_idx), np.asarray(row_ids))

    # weight host layout [p, n, kt, j] = Wt[(kt0+kt)*128+p, n*512+j]
    wtr = wt.reshape(K_TILES, P, N_BLOCKS, NB)
    wt8_l = wtr[:F8].transpose(1, 2, 0, 3).astype(NP_FP8)
    wtb_l = wtr[F8:].transpose(1, 2, 0, 3).astype(NP_BF16)

    xf = x.reshape(M_TOT, N_COLS)
    in_maps = []
    for c in range(N_CORES):
        xsh = xf[c * M : (c + 1) * M]  # [1024, 4096]
        # xsT[kt, p, m] = xsh[m, kt*128+p]
        xsT = np.ascontiguousarray(xsh.T).reshape(K_TILES, P, M)
        xs8 = xsT[:F8].transpose(1, 0, 2).astype(NP_FP8)
        xsb = xsT[F8:].transpose(1, 0, 2).astype(NP_BF16)
        in_maps.append({"xs8": xs8, "xsb": xsb, "wt8": wt8_l, "wtb": wtb_l})

    nc = _get_nc()
    res = run_bass_kernel_spmd(
        nc, in_maps, core_ids=list(range(N_CORES)), trace=trace
    )
    LAST_EXEC_NS = res.exec_time_ns

    out = np.concatenate([r["out"] for r in res.results], axis=0)
    return out.reshape(B, S, N_ROWS)
